# revision 4
# baseline (speedup 1.0000x reference)
"""Trainium2 Bass kernel for nn_Classifier (GNN edge-MLP link predictor).

Computes, for E candidate edges:
    out[e] = W2 . relu( x_nc[i0[e]] @ W1[:H] + x_pr[i1[e]] @ W1[H:] + b1 ) + b2

The naive per-edge-gather kernel is GPSIMD descriptor-generation bound
(~7.5 ns/index on the Pool engine, measured on HW), not DMA bound. This
kernel removes one of the two per-edge gathers entirely and keeps the
other as the sole Pool-engine user:

  1. fc1 splits per node table:  z[e] = P0[i0[e]] + P1[i1[e]] + b1,
     with P0 = x_nc @ W1[:H], P1 = x_pr @ W1[H:] precomputed per core
     on the tensor engine (2 x 20k x 128 x 128 matmuls, ~40us).

  2. The i0 side costs ZERO Pool time: edges are bucketed by i0-block
     (i0//128) on the host and load-balanced across the 8 cores so each
     (core, block) holds <= 832 edges; each block gets a fixed 832-col
     slot (uniform compile-time schedule for the single SPMD program).
     P0 rows are expanded into the per-edge stream by the tensor engine
     multiplying a streamed fp8 {0,1} one-hot matrix (content is
     runtime data, structure is compile time).

  3. The i1 side uses SBUF-source dma_gather from P1 (node-major in
     SBUF): 16 chunks x 8192 indices -> ~1.0 ms Pool critical path.

  4. z = (stream + b1) + g1 on DVE, relu on ScalarE, fc2 dot with W2 on
     the tensor engine; host un-permutes the scalar outputs.

Tables bf16, f32 PSUM accumulation.
"""

import numpy as np
import ml_dtypes

import concourse.bass as bass
import concourse.tile as tile
from concourse import bacc, mybir
from concourse import bass_utils

F32 = mybir.dt.float32
BF16 = mybir.dt.bfloat16
I16 = mybir.dt.int16

N_CORES = 8
H = 128

# Full-problem geometry (hardcoded per the task contract).
E_TOTAL = 1_000_000
N_NODES = 20_000

NB = 157                  # node blocks of 128
N_PAD = NB * 128          # 20096
SLOT = 832                # columns per (core, node-block); host asserts fit
GRID = NB * SLOT          # 130624
TG = 8192                 # indices per dma_gather chunk
N_CHUNKS = 16
E_PAD = N_CHUNKS * TG     # 131072 (>= GRID)
TT = 512                  # compute tile (psum-bank width)
XC = 4096                 # x-table columns per precompute chunk

# one-hot dtype: "fp8" (1B, matmul vs bf16 stat), "bf16" (2B, safe)
OH_MODE = "fp8"


def _build():
    nc = bacc.Bacc(
        "TRN2", target_bir_lowering=False, debug=False, num_devices=N_CORES
    )
    oh_dt = mybir.dt.float8e4 if OH_MODE == "fp8" else BF16

    xn = nc.dram_tensor("xn", [H, N_PAD], BF16, kind="ExternalInput").ap()
    xp = nc.dram_tensor("xp", [H, N_PAD], BF16, kind="ExternalInput").ap()
    w1a = nc.dram_tensor("w1a", [H, H], BF16, kind="ExternalInput").ap()
    w1b = nc.dram_tensor("w1b", [H, H], BF16, kind="ExternalInput").ap()
    b1 = nc.dram_tensor("b1", [H, 1], F32, kind="ExternalInput").ap()
    w2 = nc.dram_tensor("w2", [H, 1], BF16, kind="ExternalInput").ap()
    b2 = nc.dram_tensor("b2", [1, 1], F32, kind="ExternalInput").ap()
    oh = nc.dram_tensor("oh", [128, E_PAD], oh_dt, kind="ExternalInput").ap()
    idx1 = nc.dram_tensor("idx1", [16, E_PAD // 16], I16, kind="ExternalInput").ap()
    out = nc.dram_tensor("out", [1, E_PAD], F32, kind="ExternalOutput").ap()

    relu = mybir.ActivationFunctionType.Relu
    ident = mybir.ActivationFunctionType.Identity
    add_op = mybir.AluOpType.add

    with tile.TileContext(nc) as tc:
        with (
            tc.tile_pool(name="const", bufs=1) as cpool,
            tc.tile_pool(name="xs", bufs=2) as xpool,
            tc.tile_pool(name="ohp", bufs=4) as ohpool,
            tc.tile_pool(name="g", bufs=2) as gpool,
            tc.tile_pool(name="zh", bufs=3) as zpool,
            tc.tile_pool(name="stg", bufs=6) as spool,
            tc.tile_pool(name="pre", bufs=2, space="PSUM") as prepool,
            tc.tile_pool(name="stream", bufs=3, space="PSUM") as stpool,
            tc.tile_pool(name="fc2", bufs=2, space="PSUM") as fc2pool,
        ):
            # ---- constants ----
            w1a_sb = cpool.tile([H, H], BF16, tag="w1a")
            nc.sync.dma_start(w1a_sb[:], w1a[:])
            w1b_sb = cpool.tile([H, H], BF16, tag="w1b")
            nc.sync.dma_start(w1b_sb[:], w1b[:])
            b1_sb = cpool.tile([H, 1], F32, tag="b1")
            nc.sync.dma_start(b1_sb[:], b1[:])
            w2_sb = cpool.tile([H, 1], BF16, tag="w2")
            nc.sync.dma_start(w2_sb[:], w2[:])
            b2_sb = cpool.tile([1, 1], F32, tag="b2")
            nc.sync.dma_start(b2_sb[:], b2[:])

            idx_sb = cpool.tile([128, E_PAD // 16], I16, tag="idx")
            for k in range(8):
                nc.scalar.dma_start(idx_sb[16 * k : 16 * (k + 1), :], idx1[:])

            # node-major projected tables:
            #   p0[p, 128k + h] = P0[128k + p, h]   (matmul stationary slices)
            #   p1[p, k, h]     = P1[128k + p, h]   (SBUF-source gather tokens)
            p0 = cpool.tile([128, NB * H], BF16, tag="p0")
            p1 = cpool.tile([128, NB, H], BF16, tag="p1")

            # ---- precompute P0 / P1 on the tensor engine ----
            for tab, w_sb, which in ((xn, w1a_sb, 0), (xp, w1b_sb, 1)):
                for ci in range((N_PAD + XC - 1) // XC):
                    lo = ci * XC
                    cw = min(XC, N_PAD - lo)
                    xs = xpool.tile([128, XC], BF16, tag="xs")
                    nc.scalar.dma_start(xs[:, :cw], tab[:, lo : lo + cw])
                    for bl in range(cw // H):
                        k = lo // H + bl
                        ps = prepool.tile([128, H], F32, tag="pre")
                        nc.tensor.matmul(
                            ps[:],
                            xs[:, bl * H : (bl + 1) * H],
                            w_sb[:],
                            start=True,
                            stop=True,
                        )
                        dst = (
                            p0[:, k * H : (k + 1) * H]
                            if which == 0
                            else p1[:, k, :]
                        )
                        if k % 2 == 0:
                            nc.scalar.copy(dst, ps[:])
                        else:
                            nc.vector.tensor_scalar(
                                dst, ps[:], 0.0, None, add_op
                            )

            # ---- main loop ----
            oh_engines = (nc.sync, nc.scalar)
            for c in range(N_CHUNKS):
                g1 = gpool.tile([128, TG], BF16, tag="g1")
                nc.gpsimd.dma_gather(
                    g1[:].rearrange("p (one t) -> p one t", one=1),
                    p1[:],
                    idx_sb[:, c * (TG // 16) : (c + 1) * (TG // 16)],
                    TG,
                    TG,
                    H,
                    transpose=True,
                    single_packet=False,
                    sbuf_tokens_per_rank=128,
                    sbuf_free_dim_per_rank=H * 2,
                )
                for tl in range(TG // TT):
                    t = c * (TG // TT) + tl
                    lo = t * TT
                    oh_t = ohpool.tile([128, TT], oh_dt, tag="oh")
                    oh_engines[t % 2].dma_start(oh_t[:], oh[:, lo : lo + TT])

                    ps = stpool.tile([128, TT], F32, tag="stream")
                    k0 = min(lo // SLOT, NB - 1)
                    k1 = min((lo + TT - 1) // SLOT, NB - 1)
                    if k0 == k1:
                        nc.tensor.matmul(
                            ps[:],
                            p0[:, k0 * H : (k0 + 1) * H],
                            oh_t[:],
                            start=True,
                            stop=True,
                        )
                    else:
                        b = SLOT * k1 - lo
                        nc.tensor.matmul(
                            ps[:, :b],
                            p0[:, k0 * H : (k0 + 1) * H],
                            oh_t[:, :b],
                            start=True,
                            stop=True,
                        )
                        nc.tensor.matmul(
                            ps[:, b:],
                            p0[:, k1 * H : (k1 + 1) * H],
                            oh_t[:, b:],
                            start=True,
                            stop=True,
                        )

                    # z = (stream + b1) + g1 ; h = relu(z)
                    z = zpool.tile([128, TT], BF16, tag="z")
                    nc.vector.scalar_tensor_tensor(
                        z[:],
                        ps[:],
                        b1_sb[:],
                        g1[:, tl * TT : (tl + 1) * TT],
                        add_op,
                        add_op,
                    )
                    h = zpool.tile([128, TT], BF16, tag="h")
                    nc.scalar.activation(h[:], z[:], relu)

                    ps2 = fc2pool.tile([1, TT], F32, tag="fc2")
                    nc.tensor.matmul(ps2[:], w2_sb[:], h[:], start=True, stop=True)
                    stage = spool.tile([1, TT], F32, tag="stage")
                    if tl % 2 == 0:
                        nc.vector.tensor_scalar(
                            stage[:], ps2[:], b2_sb[:], None, add_op
                        )
                    else:
                        nc.scalar.activation(
                            stage[:], ps2[:], ident, bias=b2_sb[:]
                        )
                    oh_engines[(t + 1) % 2].dma_start(
                        out[:, lo : lo + TT], stage[:]
                    )

    nc.compile()
    return nc


# ---------------------------------------------------------------------------
# Host-side wrapper
# ---------------------------------------------------------------------------

_CACHE: dict = {}


def _get_program():
    if "prog" not in _CACHE:
        _CACHE["prog"] = _build()
    return _CACHE["prog"]


def kernel(
    x_ncRNA: np.ndarray,
    x_Protein: np.ndarray,
    edge_label_index: np.ndarray,
    W1: np.ndarray,
    b1: np.ndarray,
    W2: np.ndarray,
    b2: np.ndarray,
    _trace: bool = False,
) -> np.ndarray:
    E = edge_label_index.shape[1]
    n_nodes = x_ncRNA.shape[0]
    assert n_nodes <= N_PAD

    nc = _get_program()

    def fm(x):  # [n, H] f32 -> feature-major bf16 [H, N_PAD]
        t = np.zeros((H, N_PAD), ml_dtypes.bfloat16)
        t[:, :n_nodes] = np.ascontiguousarray(x.astype(ml_dtypes.bfloat16).T)
        return t

    xn = fm(x_ncRNA)
    xp = fm(x_Protein)
    w1a = np.ascontiguousarray(W1[:H].astype(ml_dtypes.bfloat16))
    w1b = np.ascontiguousarray(W1[H:].astype(ml_dtypes.bfloat16))
    w2_ = np.ascontiguousarray(W2.reshape(H, 1).astype(ml_dtypes.bfloat16))
    b1_ = np.ascontiguousarray(b1.reshape(H, 1).astype(np.float32))
    b2_ = np.ascontiguousarray(b2.reshape(1, 1).astype(np.float32))
    oh_np_dt = ml_dtypes.float8_e4m3 if OH_MODE == "fp8" else ml_dtypes.bfloat16

    # ---- bucket edges by i0-block, load-balance blocks across cores ----
    i0 = np.asarray(edge_label_index[0]).astype(np.int64)
    i1 = np.asarray(edge_label_index[1]).astype(np.int64)
    blocks = i0 // 128
    order = np.argsort(blocks, kind="stable")
    bs = blocks[order]
    cnt = np.bincount(bs, minlength=NB)
    assert cnt.max() <= SLOT * N_CORES, f"block overflow: {cnt.max()}"
    starts = np.concatenate(([0], np.cumsum(cnt)))[:-1]
    grank = np.arange(E) - starts[bs]
    core_of = grank % N_CORES
    rank = grank // N_CORES
    cols_all = bs * SLOT + rank

    in_maps = []
    unmaps = []
    for c in range(N_CORES):
        m = core_of == c
        eid = order[m]
        cols = cols_all[m]
        idxp = np.zeros(E_PAD, np.int16)
        idxp[cols] = i1[eid].astype(np.int16)
        ohm = np.zeros((128, E_PAD), oh_np_dt)
        ohm[i0[eid] % 128, cols] = 1.0
        in_maps.append(
            {
                "xn": xn,
                "xp": xp,
                "w1a": w1a,
                "w1b": w1b,
                "b1": b1_,
                "w2": w2_,
                "b2": b2_,
                "oh": ohm,
                "idx1": np.ascontiguousarray(idxp.reshape(E_PAD // 16, 16).T),
            }
        )
        unmaps.append((eid, cols))

    res = bass_utils.run_bass_kernel_spmd(
        nc, in_maps, core_ids=list(range(N_CORES)), trace=_trace
    )
    out = np.empty(E, np.float32)
    for c in range(N_CORES):
        eid, cols = unmaps[c]
        out[eid] = res.results[c]["out"][0][cols]
    kernel._last_results = res
    return out


# revision 6
# speedup vs baseline: 1.0438x; 1.0438x over previous
"""Trainium2 Bass kernel for nn_Classifier (GNN edge-MLP link predictor).

Computes, for E candidate edges:
    out[e] = W2 . relu( x_nc[i0[e]] @ W1[:H] + x_pr[i1[e]] @ W1[H:] + b1 ) + b2

The naive per-edge-gather kernel is GPSIMD descriptor-generation bound
(~7.5 ns/index on the Pool engine, measured on HW), not DMA bound. This
kernel removes one of the two per-edge gathers entirely and keeps the
other as the sole Pool-engine user:

  1. fc1 splits per node table:  z[e] = P0[i0[e]] + P1[i1[e]] + b1,
     with P0 = x_nc @ W1[:H], P1 = x_pr @ W1[H:] precomputed per core
     on the tensor engine (2 x 20k x 128 x 128 matmuls, ~40us).

  2. The i0 side costs ZERO Pool time: edges are bucketed by i0-block
     (i0//128) on the host and load-balanced across the 8 cores so each
     (core, block) holds <= 832 edges; each block gets a fixed 832-col
     slot (uniform compile-time schedule for the single SPMD program).
     P0 rows are expanded into the per-edge stream by the tensor engine
     multiplying a streamed fp8 {0,1} one-hot matrix (content is
     runtime data, structure is compile time).

  3. The i1 side uses SBUF-source dma_gather from P1 (node-major in
     SBUF): 16 chunks x 8192 indices -> ~1.0 ms Pool critical path.

  4. z = (stream + b1) + g1 on DVE, relu on ScalarE, fc2 dot with W2 on
     the tensor engine; host un-permutes the scalar outputs.

Tables bf16, f32 PSUM accumulation.
"""

import numpy as np
import ml_dtypes

import concourse.bass as bass
import concourse.tile as tile
from concourse import bacc, mybir
from concourse import bass_utils

F32 = mybir.dt.float32
BF16 = mybir.dt.bfloat16
I16 = mybir.dt.int16

N_CORES = 8
H = 128

# Full-problem geometry (hardcoded per the task contract).
E_TOTAL = 1_000_000
N_NODES = 20_000

NB = 157                  # node blocks of 128
N_PAD = NB * 128          # 20096
SLOT = 832                # columns per (core, node-block); host asserts fit
GRID = NB * SLOT          # 130624
TG = 8192                 # indices per dma_gather chunk
N_CHUNKS = 16
E_PAD = N_CHUNKS * TG     # 131072 (>= GRID)
TT = 512                  # compute tile (psum-bank width)
XC = 4096                 # x-table columns per precompute chunk

# one-hot dtype: "fp8" (1B, matmul vs bf16 stat), "bf16" (2B, safe)
OH_MODE = "fp8"


def _build():
    nc = bacc.Bacc(
        "TRN2",
        target_bir_lowering=False,
        debug=False,
        num_devices=N_CORES,
        dynamic_dma_scratch_size=32768,
    )
    oh_dt = mybir.dt.float8e4 if OH_MODE == "fp8" else BF16

    xn = nc.dram_tensor("xn", [H, N_PAD], BF16, kind="ExternalInput").ap()
    xp = nc.dram_tensor("xp", [H, N_PAD], BF16, kind="ExternalInput").ap()
    w1a = nc.dram_tensor("w1a", [H, H], BF16, kind="ExternalInput").ap()
    w1b = nc.dram_tensor("w1b", [H, H], BF16, kind="ExternalInput").ap()
    b1 = nc.dram_tensor("b1", [H, 1], F32, kind="ExternalInput").ap()
    w2 = nc.dram_tensor("w2", [H, 1], BF16, kind="ExternalInput").ap()
    b2 = nc.dram_tensor("b2", [1, 1], F32, kind="ExternalInput").ap()
    oh = nc.dram_tensor("oh", [128, E_PAD], oh_dt, kind="ExternalInput").ap()
    idx1 = nc.dram_tensor("idx1", [16, E_PAD // 16], I16, kind="ExternalInput").ap()
    out = nc.dram_tensor("out", [1, E_PAD], F32, kind="ExternalOutput").ap()

    relu = mybir.ActivationFunctionType.Relu
    ident = mybir.ActivationFunctionType.Identity
    add_op = mybir.AluOpType.add

    with tile.TileContext(nc) as tc:
        with (
            tc.tile_pool(name="const", bufs=1) as cpool,
            tc.tile_pool(name="xs", bufs=2) as xpool,
            tc.tile_pool(name="ohp", bufs=4) as ohpool,
            tc.tile_pool(name="g", bufs=2) as gpool,
            tc.tile_pool(name="zh", bufs=3) as zpool,
            tc.tile_pool(name="stg", bufs=6) as spool,
            tc.tile_pool(name="pre", bufs=2, space="PSUM") as prepool,
            tc.tile_pool(name="stream", bufs=3, space="PSUM") as stpool,
            tc.tile_pool(name="fc2", bufs=2, space="PSUM") as fc2pool,
        ):
            # ---- constants ----
            w1a_sb = cpool.tile([H, H], BF16, tag="w1a")
            nc.sync.dma_start(w1a_sb[:], w1a[:])
            w1b_sb = cpool.tile([H, H], BF16, tag="w1b")
            nc.sync.dma_start(w1b_sb[:], w1b[:])
            b1_sb = cpool.tile([H, 1], F32, tag="b1")
            nc.sync.dma_start(b1_sb[:], b1[:])
            w2_sb = cpool.tile([H, 1], BF16, tag="w2")
            nc.sync.dma_start(w2_sb[:], w2[:])
            b2_sb = cpool.tile([1, 1], F32, tag="b2")
            nc.sync.dma_start(b2_sb[:], b2[:])

            idx_sb = cpool.tile([128, E_PAD // 16], I16, tag="idx")
            for k in range(8):
                nc.scalar.dma_start(idx_sb[16 * k : 16 * (k + 1), :], idx1[:])

            # node-major projected tables:
            #   p0[p, 128k + h] = P0[128k + p, h]   (matmul stationary slices)
            #   p1[p, k, h]     = P1[128k + p, h]   (SBUF-source gather tokens)
            p0 = cpool.tile([128, NB * H], BF16, tag="p0")
            p1 = cpool.tile([128, NB, H], BF16, tag="p1")

            # ---- precompute P0 / P1 on the tensor engine ----
            for tab, w_sb, which in ((xp, w1b_sb, 1), (xn, w1a_sb, 0)):
                for ci in range((N_PAD + XC - 1) // XC):
                    lo = ci * XC
                    cw = min(XC, N_PAD - lo)
                    xs = xpool.tile([128, XC], BF16, tag="xs")
                    (nc.sync if ci % 2 == 0 else nc.scalar).dma_start(
                        xs[:, :cw], tab[:, lo : lo + cw]
                    )
                    for bl in range(cw // H):
                        k = lo // H + bl
                        ps = prepool.tile([128, H], F32, tag="pre")
                        nc.tensor.matmul(
                            ps[:],
                            xs[:, bl * H : (bl + 1) * H],
                            w_sb[:],
                            start=True,
                            stop=True,
                        )
                        dst = (
                            p0[:, k * H : (k + 1) * H]
                            if which == 0
                            else p1[:, k, :]
                        )
                        if k % 2 == 0:
                            nc.scalar.copy(dst, ps[:])
                        else:
                            nc.vector.tensor_scalar(
                                dst, ps[:], 0.0, None, add_op
                            )

            # ---- main loop ----
            oh_engines = (nc.sync, nc.scalar)
            for c in range(N_CHUNKS):
                g1 = gpool.tile([128, TG], BF16, tag="g1")
                nc.gpsimd.dma_gather(
                    g1[:].rearrange("p (one t) -> p one t", one=1),
                    p1[:],
                    idx_sb[:, c * (TG // 16) : (c + 1) * (TG // 16)],
                    TG,
                    TG,
                    H,
                    transpose=True,
                    single_packet=False,
                    sbuf_tokens_per_rank=128,
                    sbuf_free_dim_per_rank=H * 2,
                )
                for tl in range(TG // TT):
                    t = c * (TG // TT) + tl
                    lo = t * TT
                    oh_t = ohpool.tile([128, TT], oh_dt, tag="oh")
                    oh_engines[t % 2].dma_start(oh_t[:], oh[:, lo : lo + TT])

                    ps = stpool.tile([128, TT], F32, tag="stream")
                    k0 = min(lo // SLOT, NB - 1)
                    k1 = min((lo + TT - 1) // SLOT, NB - 1)
                    if k0 == k1:
                        nc.tensor.matmul(
                            ps[:],
                            p0[:, k0 * H : (k0 + 1) * H],
                            oh_t[:],
                            start=True,
                            stop=True,
                        )
                    else:
                        b = SLOT * k1 - lo
                        nc.tensor.matmul(
                            ps[:, :b],
                            p0[:, k0 * H : (k0 + 1) * H],
                            oh_t[:, :b],
                            start=True,
                            stop=True,
                        )
                        nc.tensor.matmul(
                            ps[:, b:],
                            p0[:, k1 * H : (k1 + 1) * H],
                            oh_t[:, b:],
                            start=True,
                            stop=True,
                        )

                    # z = (stream + b1) + g1 ; h = relu(z)
                    z = zpool.tile([128, TT], BF16, tag="z")
                    nc.vector.scalar_tensor_tensor(
                        z[:],
                        ps[:],
                        b1_sb[:],
                        g1[:, tl * TT : (tl + 1) * TT],
                        add_op,
                        add_op,
                    )
                    h = zpool.tile([128, TT], BF16, tag="h")
                    nc.scalar.activation(h[:], z[:], relu)

                    ps2 = fc2pool.tile([1, TT], F32, tag="fc2")
                    nc.tensor.matmul(ps2[:], w2_sb[:], h[:], start=True, stop=True)
                    stage = spool.tile([1, TT], F32, tag="stage")
                    if tl % 2 == 0:
                        nc.vector.tensor_scalar(
                            stage[:], ps2[:], b2_sb[:], None, add_op
                        )
                    else:
                        nc.scalar.activation(
                            stage[:], ps2[:], ident, bias=b2_sb[:]
                        )
                    oh_engines[(t + 1) % 2].dma_start(
                        out[:, lo : lo + TT], stage[:]
                    )

    nc.compile()
    return nc


# ---------------------------------------------------------------------------
# Host-side wrapper
# ---------------------------------------------------------------------------

_CACHE: dict = {}


def _get_program():
    if "prog" not in _CACHE:
        _CACHE["prog"] = _build()
    return _CACHE["prog"]


def kernel(
    x_ncRNA: np.ndarray,
    x_Protein: np.ndarray,
    edge_label_index: np.ndarray,
    W1: np.ndarray,
    b1: np.ndarray,
    W2: np.ndarray,
    b2: np.ndarray,
    _trace: bool = False,
) -> np.ndarray:
    E = edge_label_index.shape[1]
    n_nodes = x_ncRNA.shape[0]
    assert n_nodes <= N_PAD

    nc = _get_program()

    def fm(x):  # [n, H] f32 -> feature-major bf16 [H, N_PAD]
        t = np.zeros((H, N_PAD), ml_dtypes.bfloat16)
        t[:, :n_nodes] = np.ascontiguousarray(x.astype(ml_dtypes.bfloat16).T)
        return t

    xn = fm(x_ncRNA)
    xp = fm(x_Protein)
    w1a = np.ascontiguousarray(W1[:H].astype(ml_dtypes.bfloat16))
    w1b = np.ascontiguousarray(W1[H:].astype(ml_dtypes.bfloat16))
    w2_ = np.ascontiguousarray(W2.reshape(H, 1).astype(ml_dtypes.bfloat16))
    b1_ = np.ascontiguousarray(b1.reshape(H, 1).astype(np.float32))
    b2_ = np.ascontiguousarray(b2.reshape(1, 1).astype(np.float32))
    oh_np_dt = ml_dtypes.float8_e4m3 if OH_MODE == "fp8" else ml_dtypes.bfloat16

    # ---- bucket edges by i0-block, load-balance blocks across cores ----
    i0 = np.asarray(edge_label_index[0]).astype(np.int64)
    i1 = np.asarray(edge_label_index[1]).astype(np.int64)
    blocks = i0 // 128
    order = np.argsort(blocks, kind="stable")
    bs = blocks[order]
    cnt = np.bincount(bs, minlength=NB)
    assert cnt.max() <= SLOT * N_CORES, f"block overflow: {cnt.max()}"
    starts = np.concatenate(([0], np.cumsum(cnt)))[:-1]
    grank = np.arange(E) - starts[bs]
    core_of = grank % N_CORES
    rank = grank // N_CORES
    cols_all = bs * SLOT + rank

    in_maps = []
    unmaps = []
    for c in range(N_CORES):
        m = core_of == c
        eid = order[m]
        cols = cols_all[m]
        idxp = np.zeros(E_PAD, np.int16)
        idxp[cols] = i1[eid].astype(np.int16)
        ohm = np.zeros((128, E_PAD), oh_np_dt)
        ohm[i0[eid] % 128, cols] = 1.0
        in_maps.append(
            {
                "xn": xn,
                "xp": xp,
                "w1a": w1a,
                "w1b": w1b,
                "b1": b1_,
                "w2": w2_,
                "b2": b2_,
                "oh": ohm,
                "idx1": np.ascontiguousarray(idxp.reshape(E_PAD // 16, 16).T),
            }
        )
        unmaps.append((eid, cols))

    res = bass_utils.run_bass_kernel_spmd(
        nc, in_maps, core_ids=list(range(N_CORES)), trace=_trace
    )
    out = np.empty(E, np.float32)
    for c in range(N_CORES):
        eid, cols = unmaps[c]
        out[eid] = res.results[c]["out"][0][cols]
    kernel._last_results = res
    return out
